# revision 14
# baseline (speedup 1.0000x reference)
"""DigitCaps dynamic-routing kernel for 8 Trainium2 NeuronCores.

Strategy (I-sharded, matching the problem's mesh_spec "i"):
  - Shard the input-capsule dim I=2048 across 8 cores (I_loc=256/core);
    W is sharded the same way, u is sliced on i. The only cross-core
    communication is an AllReduce of the tiny routing sums
    s[b,c,o] (one per routing round).
  - u_hat[b,i,c,o] = sum_d W[i,c,o,d] u[b,i,d] is materialized per
    32-batch chunk on the PE using 32x32 array tiling: weights are a
    host-prepped block-diagonal arrangement of u (4 i-values x 8 d on
    K=32), rhs is a W arrangement streaming all 160 (c,o) columns.
  - Routing (3 iters, first iter folded into a plain matmul since the
    initial softmax is uniform): per chunk, fused DVE passes compute
    the agreement dots and softmax, the weighted i-sum is reduced with
    a constant "gather" matmul on the PE, then AllReduce + squash.
"""

import sys

if "/opt/trn_rl_repo" not in sys.path:
    sys.path.insert(0, "/opt/trn_rl_repo")

import numpy as np

import concourse.bass as bass
import concourse.mybir as mybir
import concourse.tile as tile
from concourse import bass_utils
# problem constants (hardcoded per harness contract)
B, I, C, O, D = 256, 2048, 10, 16, 8
NCORES = 8
IL = I // NCORES       # 256 i per core
G = IL // 4            # 64 groups of 4 i
G4 = G // 4            # 16 group-slots per PE row-group
BC = 32                # batch chunk
NCH = B // BC          # 8 chunks
CO = C * O             # 160
KK = IL * D // 128     # 16 contraction chunks for the s0 matmul
F32 = mybir.dt.float32
AF = mybir.ActivationFunctionType
ALU = mybir.AluOpType

MAX_WAITS = 1  # this walrus build rejects multiple sem waits per instruction
NOP_WAITS = 1  # waits we pack on each injected NOP


def _max_waits_for(inst):
    return MAX_WAITS


def _cap_waits(nc):
    """Hoist excess sem waits from any instruction onto same-engine NOPs
    inserted immediately before it (semantics preserved: the engine still
    waits for every sem before executing the instruction)."""
    for f in nc.m.functions:
        for bb in f.blocks:
            insts = list(bb.instructions)
            out, changed = [], False
            for inst in insts:
                si = inst.sync_info
                waits = list(si.on_wait) if si and si.on_wait else []
                lim = _max_waits_for(inst)
                if len(waits) > lim:
                    keep = waits[-lim:]
                    rest = waits[:-lim]
                    for j in range(0, len(rest), NOP_WAITS):
                        nop = mybir.InstNoOp(
                            name=f"{inst.name}_wsplit{j}", engine=inst.engine,
                            sync_info=mybir.SyncInfo(
                                on_wait=rest[j:j + NOP_WAITS], on_update=[]))
                        out.append(nop)
                    si.on_wait = keep
                    changed = True
                out.append(inst)
            if changed:
                bb.instructions[:] = out


def prep_inputs(u, W):
    """Host-side layout prep. Returns per-core input dicts."""
    u = np.asarray(u, np.float32)
    W = np.asarray(W, np.float32)
    maps = []
    for k in range(NCORES):
        i0 = k * IL
        Wl = W[i0:i0 + IL]                    # [IL, C, O, D]
        ul = u[:, i0:i0 + IL, :]              # [B, IL, D]

        # rows (i_local, d) -> (kk, p): row = kk*128 + p
        Wrow = Wl.transpose(0, 3, 1, 2).reshape(IL * D, CO)   # [(i,d), (c,o)]
        W_sb = Wrow.reshape(KK, 128, CO).transpose(1, 0, 2).copy()  # [128, KK, CO]
        urow = ul.transpose(1, 2, 0).reshape(IL * D, B)       # [(i,d), b]
        u_s0 = urow.reshape(KK, 128, B).transpose(1, 0, 2).copy()   # [128, KK, B]

        # i_local(g4, r, i_sub) = 16*g4 + 4*r + i_sub ; partition = 32r+8is+d
        # W_mat[32r+8is+d, g4, co] = Wl[i_local, c, o, d]
        Wm = Wl.transpose(0, 3, 1, 2).reshape(G4, 4, 4, D, CO)  # [g4, r, is, d, co]
        W_mat = np.ascontiguousarray(
            Wm.transpose(1, 2, 3, 0, 4).reshape(128, G4, CO))   # [(r,is,d), g4, co]

        # u_bdw[32r+8is+d, cc, bo, g4, m=4*b8+ip] = (is==ip)*u[b, i_local(g4,r,ip), d]
        #   b = 32*cc + 8*bo + b8
        ub = ul.reshape(NCH, 4, 8, IL, D)                      # [cc, bo, b8, i, d]
        ub = ub.reshape(NCH, 4, 8, G4, 4, 4, D)                # [cc,bo,b8,g4,r,ip,d]
        u_bdw = np.zeros((4, 4, D, NCH, 4, G4, 8, 4), np.float32)  # [r,is,d,cc,bo,g4,b8,ip]
        for isb in range(4):
            # diagonal block: is == ip. ub[:,:,:,:,:,isb,:] axes:
            # [cc, bo, b8, g4, r, d] -> want [r, d, cc, bo, g4, b8]
            u_bdw[:, isb, :, :, :, :, :, isb] = (
                ub[:, :, :, :, :, isb, :].transpose(4, 5, 0, 1, 3, 2))
        u_bdw = u_bdw.reshape(128, NCH, 4, G4, 32).copy()

        # E[p=32bo+4b8+ip, m=8bo+b8] = 1
        E = np.zeros((128, 32), np.float32)
        for bo in range(4):
            for b8 in range(8):
                for ip in range(4):
                    E[32 * bo + 4 * b8 + ip, 8 * bo + b8] = 1.0

        maps.append({
            "W_sb": W_sb, "u_s0": u_s0, "W_mat": W_mat,
            "u_bdw": u_bdw, "E": E,
        })
    return maps


def build_program():
    nc = bass.Bass(num_devices=NCORES)
    W_sb_d = nc.dram_tensor("W_sb", [128, KK, CO], F32, kind="ExternalInput")
    u_s0_d = nc.dram_tensor("u_s0", [128, KK, B], F32, kind="ExternalInput")
    W_mat_d = nc.dram_tensor("W_mat", [128, G4, CO], F32, kind="ExternalInput")
    u_bdw_d = nc.dram_tensor("u_bdw", [128, NCH, 4, G4, 32], F32, kind="ExternalInput")
    E_d = nc.dram_tensor("E", [128, 32], F32, kind="ExternalInput")
    y_d = nc.dram_tensor("y", [B, CO], F32, kind="ExternalOutput")

    with tile.TileContext(nc, num_cores=NCORES) as tc:
        _emit(nc, tc, W_sb_d, u_s0_d, W_mat_d, u_bdw_d, E_d, y_d)
    _cap_waits(nc)
    return nc


def _squash(nc, pool, s_c, scale):
    """s_c: [BC, C, O] sbuf. Returns v_c [BC, C, O]. v = squash(scale*s)."""
    ssq = pool.tile([BC, C, O], F32, tag="ssq")
    nc.scalar.activation(ssq[:], s_c[:], AF.Square, scale=float(scale))
    sq = pool.tile([BC, C], F32, tag="sq")
    nc.vector.tensor_reduce(sq[:], ssq[:], axis=mybir.AxisListType.X, op=ALU.add)
    t1 = pool.tile([BC, C], F32, tag="t1")
    nc.vector.tensor_scalar_add(t1[:], sq[:], 1.0)
    r1 = pool.tile([BC, C], F32, tag="r1")
    nc.vector.reciprocal(r1[:], t1[:])
    epsb = pool.tile([BC, 1], F32, tag="epsb")
    nc.vector.memset(epsb[:], 1e-9)
    q = pool.tile([BC, C], F32, tag="q")
    nc.scalar.activation(q[:], sq[:], AF.Sqrt, bias=epsb[:])
    r2 = pool.tile([BC, C], F32, tag="r2")
    nc.vector.reciprocal(r2[:], q[:])
    coef = pool.tile([BC, C], F32, tag="coef")
    nc.vector.tensor_tensor(coef[:], r1[:], r2[:], op=ALU.mult)
    coef2 = pool.tile([BC, C], F32, tag="coef2")
    nc.vector.tensor_tensor(coef2[:], coef[:], sq[:], op=ALU.mult)
    v_c = pool.tile([BC, C, O], F32, tag="v_c")
    nc.vector.scalar_tensor_tensor(
        out=v_c[:], in0=s_c[:], scalar=float(scale), in1=coef2[:].unsqueeze(2).broadcast_to([BC, C, O]),
        op0=ALU.mult, op1=ALU.mult)
    return v_c


def _emit(nc, tc, W_sb_d, u_s0_d, W_mat_d, u_bdw_d, E_d, y_d):
    from contextlib import ExitStack
    es = ExitStack()
    statics = es.enter_context(tc.tile_pool(name="statics", bufs=1))
    smalls = es.enter_context(tc.tile_pool(name="smalls", bufs=2))
    bigs = es.enter_context(tc.tile_pool(name="bigs", bufs=1))
    dram = es.enter_context(tc.tile_pool(name="dram", bufs=1, space="DRAM"))

    # ---- static loads ----
    W_sb = statics.tile([128, KK, CO], F32)
    nc.sync.dma_start(W_sb[:], W_sb_d[:])
    u_s0 = statics.tile([128, KK, B], F32)
    nc.sync.dma_start(u_s0[:], u_s0_d[:])
    W_mat = statics.tile([128, G4, CO], F32)
    nc.sync.dma_start(W_mat[:], W_mat_d[:])
    u_bdw = statics.tile([128, NCH, 4, G4, 32], F32)
    nc.sync.dma_start(u_bdw[:], u_bdw_d[:])
    E_sb = statics.tile([128, 32], F32)
    nc.sync.dma_start(E_sb[:], E_d[:])

    # dram bounce buffers for collectives
    s_in = [dram.tile([B, CO], F32, name=f"s_in{j}", tag=f"s_in{j}")
            for j in range(3)]
    s_out = [dram.tile([B, CO], F32, name=f"s_out{j}", tag=f"s_out{j}")
             for j in range(3)]
    vtmp = dram.tile([BC, CO], F32, name="vtmp", tag="vtmp")

    # ---- s0 partial: s0[b, co] = sum_(i,d) u[row, b] * W[row, co] ----
    with tc.tile_pool(name="ps0", bufs=2, space="PSUM") as ps0:
        s0_sb = smalls.tile([128, 2, CO], F32, tag="s0_sb")  # [p, b-half, co]
        for h in range(2):
            acc = ps0.tile([128, CO], F32, tag="s0acc")
            for kk in range(KK):
                nc.tensor.matmul(
                    acc[:], u_s0[:, kk, h * 128:(h + 1) * 128], W_sb[:, kk, :],
                    start=(kk == 0), stop=(kk == KK - 1))
            nc.scalar.copy(s0_sb[:, h, :], acc[:])
        nc.sync.dma_start(
            s_in[0][:].rearrange("(h p) co -> p h co", h=2), s0_sb[:])
    nc.gpsimd.collective_compute(
        "AllReduce", ALU.add, replica_groups=[list(range(NCORES))],
        ins=[s_in[0].opt()], outs=[s_out[0].opt()])

    # ---- per-chunk routing ----
    psum_mat = es.enter_context(tc.tile_pool(name="pmat", bufs=6, space="PSUM"))
    psum_s4 = es.enter_context(tc.tile_pool(name="ps4", bufs=2, space="PSUM"))

    for ch in range(NCH):
        # --- materialize u_hat for this chunk: [128=(bo,b8,ip), G, C, O] ---
        u_hat = bigs.tile([128, G, C, O], F32, tag="u_hat")
        for g in range(G):
            r, g4 = g % 4, g // 4
            pm = psum_mat.tile([128, CO], F32, tag="pm")
            for bo in range(4):
                nc.tensor.matmul(
                    pm[32 * bo:32 * bo + 32, :],
                    u_bdw[32 * r:32 * r + 32, ch, bo, g4, :],
                    W_mat[32 * r:32 * r + 32, g4, :],
                    start=True, stop=True,
                    tile_position=(32 * r, 32 * bo))
            # evacuate (alternate engines)
            dst = u_hat[:, g, :, :]
            if g % 2 == 0:
                nc.scalar.copy(dst, pm[:])
            else:
                nc.vector.tensor_copy(dst, pm[:])

        blog = smalls.tile([128, G, C], F32, tag="blog")

        for rnd in range(2):
            # --- v from s_out[rnd] ---
            s_c = smalls.tile([BC, C, O], F32, tag="s_c")
            nc.sync.dma_start(s_c[:], s_out[rnd][ch * BC:(ch + 1) * BC, :]
                              .rearrange("b (c o) -> b c o", c=C))
            v_c = _squash(nc, smalls, s_c, 0.1 if rnd == 0 else 1.0)
            nc.sync.dma_start(vtmp[:], v_c[:].rearrange("b c o -> b (c o)"))
            v_rep = smalls.tile([128, CO], F32, tag="v_rep")
            src = (vtmp[:].rearrange("(bo b8) co -> bo b8 co", bo=4)
                   .unsqueeze(2).broadcast_to([4, 8, 4, CO]))
            nc.sync.dma_start(v_rep[:], src)

            # --- a = sum_o u_hat * v ---  (G split in halves for SBUF)
            G2 = G // 2
            a_val = smalls.tile([128, G, C], F32, tag="a_val")
            for hh in range(2):
                prod = bigs.tile([128, G2, C, O], F32, tag="prod")
                nc.vector.scalar_tensor_tensor(
                    out=prod[:], in0=u_hat[:, hh * G2:(hh + 1) * G2], scalar=0.0,
                    in1=v_rep[:].rearrange("p (c o) -> p c o", c=C)
                        .unsqueeze(1).broadcast_to([128, G2, C, O]),
                    op0=ALU.bypass, op1=ALU.mult)
                nc.vector.tensor_reduce(
                    a_val[:, hh * G2:(hh + 1) * G2], prod[:],
                    axis=mybir.AxisListType.X, op=ALU.add)
            if rnd == 0:
                nc.scalar.copy(blog[:], a_val[:])
            else:
                nc.vector.tensor_tensor(blog[:], blog[:], a_val[:], op=ALU.add)

            # --- cc = softmax_c(blog) ---
            mx = smalls.tile([128, G], F32, tag="mx")
            nc.vector.tensor_reduce(mx[:], blog[:], axis=mybir.AxisListType.X,
                                    op=ALU.max)
            esrc = smalls.tile([128, G, C], F32, tag="esrc")
            nc.vector.scalar_tensor_tensor(
                out=esrc[:], in0=blog[:], scalar=0.0,
                in1=mx[:].unsqueeze(2).broadcast_to([128, G, C]),
                op0=ALU.bypass, op1=ALU.subtract)
            ev = smalls.tile([128, G, C], F32, tag="ev")
            nc.scalar.activation(ev[:], esrc[:], AF.Exp)
            esum = smalls.tile([128, G], F32, tag="esum")
            nc.vector.tensor_reduce(esum[:], ev[:], axis=mybir.AxisListType.X,
                                    op=ALU.add)
            rsum = smalls.tile([128, G], F32, tag="rsum")
            nc.vector.reciprocal(rsum[:], esum[:])
            cc_t = smalls.tile([128, G, C], F32, tag="cc_t")
            nc.vector.scalar_tensor_tensor(
                out=cc_t[:], in0=ev[:], scalar=0.0,
                in1=rsum[:].unsqueeze(2).broadcast_to([128, G, C]),
                op0=ALU.bypass, op1=ALU.mult)

            # --- s_partial = sum_i cc * u_hat ---  (G halves)
            s3 = smalls.tile([128, C, O], F32, tag="s3")
            for hh in range(2):
                prod2 = bigs.tile([128, G2, C, O], F32, tag="prod")  # reuse slot
                nc.vector.scalar_tensor_tensor(
                    out=prod2[:], in0=u_hat[:, hh * G2:(hh + 1) * G2], scalar=0.0,
                    in1=cc_t[:, hh * G2:(hh + 1) * G2]
                        .unsqueeze(3).broadcast_to([128, G2, C, O]),
                    op0=ALU.bypass, op1=ALU.mult)
                s3h = smalls.tile([128, C, O], F32, tag="s3h")
                nc.vector.tensor_reduce(
                    s3h[:], prod2[:].transpose([0, 2, 3, 1]),
                    axis=mybir.AxisListType.X, op=ALU.add)
                if hh == 0:
                    nc.scalar.copy(s3[:], s3h[:])
                else:
                    nc.vector.tensor_tensor(s3[:], s3[:], s3h[:], op=ALU.add)
            s4p = psum_s4.tile([32, CO], F32, tag="s4p")
            nc.tensor.matmul(s4p[:], E_sb[:], s3[:].rearrange("p c o -> p (c o)"),
                             start=True, stop=True)
            s4 = smalls.tile([32, CO], F32, tag="s4")
            nc.scalar.copy(s4[:], s4p[:])
            nc.sync.dma_start(s_in[rnd + 1][ch * BC:(ch + 1) * BC, :], s4[:])
            nc.gpsimd.collective_compute(
                "AllReduce", ALU.add, replica_groups=[list(range(NCORES))],
                ins=[s_in[rnd + 1][ch * BC:(ch + 1) * BC, :].opt()],
                outs=[s_out[rnd + 1][ch * BC:(ch + 1) * BC, :].opt()])

        # --- final v for this chunk ---
        s_c2 = smalls.tile([BC, C, O], F32, tag="s_c")
        nc.sync.dma_start(s_c2[:], s_out[2][ch * BC:(ch + 1) * BC, :]
                          .rearrange("b (c o) -> b c o", c=C))
        v_fin = _squash(nc, smalls, s_c2, 1.0)
        nc.sync.dma_start(y_d[ch * BC:(ch + 1) * BC, :],
                          v_fin[:].rearrange("b c o -> b (c o)"))

    es.close()


_CACHE = {}


def kernel(u, W):
    maps = prep_inputs(u, W)
    if "nc" not in _CACHE:
        _CACHE["nc"] = build_program()
    nc = _CACHE["nc"]
    res = bass_utils.run_bass_kernel_spmd(nc, maps, core_ids=list(range(NCORES)))
    y = res.results[0]["y"].reshape(B, C, O).astype(np.float32)
    return y
